# revision 87
# baseline (speedup 1.0000x reference)
"""DeepConvAE Trainium2 kernel: 3x conv5x5+relu -> block-argmax sparsify ->
3x deconv5x5 (relu, relu, sigmoid). Data-parallel over batch: 64 samples
split 8-per-core across 8 NeuronCores; all weights replicated.

Convs are computed as 25-tap matmul accumulation in PSUM with input channels
on partitions (K), output channels as the stationary M dim, and output-pixel
row-chunks as the moving free dim N (<=512 per PSUM bank). Deconvs are convs
over zero-padded inputs with spatially-flipped / transposed weights (done on
host). conv1 (3 input channels) uses an im2col tile (K = 25 taps x 3 ch = 75)
built with strided DMAs straight from HBM.
"""

import sys

sys.path.insert(0, "/opt/trn_rl_repo")

import numpy as np

import bass_rust
import ml_dtypes
import concourse.bass as bass
import concourse.mybir as mybir

_ml_bf16 = np.dtype(ml_dtypes.bfloat16)
import concourse.tile as tile
from concourse import bass_isa
from concourse import library_config

F32 = mybir.dt.float32
F32R = mybir.dt.float32r
BF16 = mybir.dt.bfloat16

N_CORES = 8
BPC = 8  # batch per core
TAPS = [(dy, dx) for dy in range(5) for dx in range(5)]

# row-chunking per layer: (Hi, Wi, Ho, Wo, chunk row counts)
# all chunk N = R*Wo kept >= 256 so float32r streams at full rate
CONV1 = (64, 64, 60, 60, [8, 8, 8, 8, 7, 7, 7, 7])
CONV2 = (60, 60, 56, 56, [8, 8, 8, 8, 8, 8, 8])
CONV3 = (56, 56, 52, 52, [8, 8, 8, 7, 7, 7, 7])
DECONV1 = (60, 60, 56, 56, [8, 8, 8, 8, 8, 8, 8])  # input = padded 52+8
DECONV2 = (64, 64, 60, 60, [8, 8, 8, 8, 7, 7, 7, 7])  # input = padded 56+8
DECONV3 = (68, 68, 64, 64, [8, 8, 8, 8, 8, 8, 8, 8])  # input = padded 60+8


def _split_waits(nc):
    """This toolchain's walrus allows at most ONE sync-wait command per engine
    instruction ("Too many sync wait commands"). Tile emits one wait per
    upstream proc, so split: insert same-engine NOPs, each carrying one of the
    extra waits, immediately before the offending instruction."""
    for f in nc.m.functions:
        for blk in f.blocks:
            new = []
            for inst in blk.instructions:
                si = inst.sync_info
                if si is not None and si.on_wait is not None and len(si.on_wait) > 1:
                    waits = list(si.on_wait)
                    for k, w in enumerate(waits[:-1]):
                        nop = mybir.InstNoOp(name=f"{inst.name}_w{k}", ins=[], outs=[])
                        nop.engine = inst.engine
                        nop.sync_info = mybir.SyncInfo(on_wait=[w], on_update=[])
                        new.append(nop)
                    inst.sync_info = mybir.SyncInfo(
                        on_wait=[waits[-1]], on_update=list(si.on_update)
                    )
                new.append(inst)
            blk.instructions = new


def _r0s(chunks):
    r0, out = 0, []
    for r in chunks:
        out.append((r0, r))
        r0 += r
    return out


def build(n_samples=BPC, mm_dtype="f32r", split_waits=True, repeat=1):
    """Build the single-core Bass program (SPMD: same program on all cores)."""
    nc = bass.Bass()
    MMD = F32R if mm_dtype == "f32r" else F32
    DD = BF16 if mm_dtype == "bf16d" else MMD  # decoder (deconv1/2) dtype

    x_d = nc.dram_tensor("x", [n_samples, 3, 64, 64], F32, kind="ExternalInput")
    w1_d = nc.dram_tensor("w1", [75, 128], F32, kind="ExternalInput")
    w2_d = nc.dram_tensor("w2", [128, 25, 128], F32, kind="ExternalInput")
    w3_d = nc.dram_tensor("w3", [128, 25, 128], F32, kind="ExternalInput")
    w4_d = nc.dram_tensor("w4", [128, 25, 128], DD, kind="ExternalInput")
    w5_d = nc.dram_tensor("w5", [128, 25, 128], DD, kind="ExternalInput")
    w6_d = nc.dram_tensor("w6", [128, 25, 3], BF16, kind="ExternalInput")
    b1_d = nc.dram_tensor("b1", [128, 1], F32, kind="ExternalInput")
    b2_d = nc.dram_tensor("b2", [128, 1], F32, kind="ExternalInput")
    b3_d = nc.dram_tensor("b3", [128, 1], F32, kind="ExternalInput")
    b4_d = nc.dram_tensor("b4", [128, 1], F32, kind="ExternalInput")
    b5_d = nc.dram_tensor("b5", [128, 1], F32, kind="ExternalInput")
    b6_d = nc.dram_tensor("b6", [99, 1], F32, kind="ExternalInput")
    eye_d = nc.dram_tensor("eye", [128, 128], F32, kind="ExternalInput")
    zc_d = nc.dram_tensor("zc", [128, 272], DD, kind="ExternalInput")
    if repeat > 1:
        csum_d = nc.dram_tensor("csum", [3, 1], F32, kind="ExternalOutput")
    out_d = nc.dram_tensor("out", [n_samples, 3, 64, 64], F32, kind="ExternalOutput")

    AF = mybir.ActivationFunctionType

    with tile.TileContext(nc) as tc:
        with (
            tc.tile_pool(name="wp", bufs=1) as wp,
            tc.tile_pool(name="act", bufs=1) as ap_,
            tc.tile_pool(name="ps", bufs=8, space=bass.MemorySpace.PSUM) as psp,
        ):
            # ---- resident weights/biases ----
            w1 = wp.tile([75, 128], F32)
            nc.sync.dma_start(w1[:, :], w1_d[:, :])
            ws = {}
            for nm, d in (("w2", w2_d), ("w3", w3_d), ("w4", w4_d), ("w5", w5_d)):
                t = wp.tile([128, 25, 128], F32 if nm in ("w2", "w3") else DD, tag=nm)
                nc.sync.dma_start(t[:, :, :], d[:, :, :])
                ws[nm] = t
            w6 = wp.tile([128, 25, 3], BF16)
            nc.sync.dma_start(w6[:, :, :], w6_d[:, :, :])
            bs = {}
            for nm, d in (("b1", b1_d), ("b2", b2_d), ("b3", b3_d), ("b4", b4_d), ("b5", b5_d)):
                t = wp.tile([128, 1], F32, tag=nm)
                nc.sync.dma_start(t[:, :], d[:, :])
                bs[nm] = t
            b6 = wp.tile([99, 1], F32)
            nc.sync.dma_start(b6[:, :], b6_d[:, :])

            # ---- persistent activation tiles (reused across samples) ----
            c1in = ap_.tile([75, 3600], F32)   # conv1 im2col (tap,ci rows)
            h1 = ap_.tile([128, 3600], F32)    # 60x60; reused as padded p1
            h2 = ap_.tile([128, 3136], F32)    # 56x56
            h3 = ap_.tile([128, 2704], F32)    # 52x52
            hm = ap_.tile([128, 2704], F32)    # spatially-masked h3
            msk = ap_.tile([128, 2704], F32)   # scratch mask
            bm1 = ap_.tile([128, 676], F32)
            bm = ap_.tile([128, 169], F32)
            mb = ap_.tile([128, 169], F32)
            rT = ap_.tile([128, 2], F32)
            mb1 = ap_.tile([1, 169], F32)
            ones1 = ap_.tile([1, 128], F32)
            rmax = ap_.tile([128, 1], F32)
            nc.vector.memset(ones1[:, :], 1.0)
            eye = wp.tile([128, 128], F32)
            nc.sync.dma_start(eye[:, :], eye_d[:, :])
            p1 = ap_.tile([128, 3600], DD)     # padded 60x60 sparsify output
            p2 = ap_.tile([128, 4096], DD)     # padded 64x64
            p3 = ap_.tile([128, 4624], BF16)   # padded 68x68 (bf16 deconv3)
            o_sb = ap_.tile([99, 1024], F32)   # (group,co) x (half, 512)

            # zero pad borders (DMA from zero DRAM: memset can't emit f32r);
            # interiors are fully rewritten per sample
            def zero_borders(pt, W, zsrc):
                pv = pt[:, :].rearrange("p (h w) -> p h w", w=W)
                nc.sync.dma_start(pt[:, 0 : 4 * W], zsrc[:, 0 : 4 * W])
                nc.sync.dma_start(pt[:, (W - 4) * W : W * W], zsrc[:, 0 : 4 * W])
                nc.sync.dma_start(pv[:, 4 : W - 4, 0:4],
                                  zsrc[:, 0 : 4 * (W - 8)].rearrange("p (h w) -> p h w", w=4))
                nc.sync.dma_start(pv[:, 4 : W - 4, W - 4 : W],
                                  zsrc[:, 0 : 4 * (W - 8)].rearrange("p (h w) -> p h w", w=4))

            zero_borders(p2, 64, zc_d[:, :])
            zero_borders(p3, 68, zc_d[:, :] if DD == BF16 else zc_d[:, :].bitcast(BF16))
            if repeat > 1:
                csum = ap_.tile([99, 2], F32)
                nc.vector.memset(csum[:, :], 0.0)

            def conv(in_tile, geom, w, bias, dst_fn, func, M=128):
                Hi, Wi, Ho, Wo, chunks = geom
                iv = in_tile[:, :].rearrange("p (h w) -> p h w", w=Wi)
                for r0, R in _r0s(chunks):
                    N = R * Wo
                    ps = psp.tile([128, 512], F32, tag="ps", name="ps")
                    for t, (dy, dx) in enumerate(TAPS):
                        rhs = iv[:, r0 + dy : r0 + dy + R, dx : dx + Wo]
                        nc.tensor.matmul(
                            ps[:, :N],
                            w[:, t, :],
                            rhs,
                            start=(t == 0),
                            stop=(t == 24),
                        )
                    nc.scalar.activation(
                        dst_fn(r0, R),
                        ps[:, :N].rearrange("p (r w) -> p r w", w=Wo),
                        func,
                        bias=bias,
                    )

            # repeat>1 is a timing aid: first run the pipeline repeat-1 extra
            # times over existing inputs, folding results into a tiny checksum
            # output (keeps the work live without extra transfers); the real
            # n_samples iterations run last.
            warm = [(r % n_samples, None) for r in range(repeat - 1)]
            for bi, b in warm + [(b, b) for b in range(n_samples)]:
                # ---- conv1: im2col (75 = tap x ch rows), one matmul/chunk ----
                for t, (dy, dx) in enumerate(TAPS):
                    nc.sync.dma_start(
                        c1in[3 * t : 3 * t + 3, :],
                        x_d[bi, :, dy : dy + 60, dx : dx + 60],
                    )
                h1v = h1[:, :].rearrange("p (h w) -> p h w", w=60)
                for r0, R in _r0s(CONV1[4]):
                    N = R * 60
                    ps = psp.tile([128, 512], F32, tag="ps", name="ps")
                    nc.tensor.matmul(ps[:, :N], w1[:, :],
                                     c1in[:, r0 * 60 : (r0 + R) * 60],
                                     start=True, stop=True)
                    nc.scalar.activation(
                        h1v[:, r0 : r0 + R, :],
                        ps[:, :N].rearrange("p (r w) -> p r w", w=60),
                        AF.Relu, bias=bs["b1"])

                # ---- conv2, conv3 ----
                h2v = h2[:, :].rearrange("p (h w) -> p h w", w=56)
                conv(h1, CONV2, ws["w2"], bs["b2"],
                     lambda r0, R: h2v[:, r0 : r0 + R, :], AF.Relu)
                h3v = h3[:, :].rearrange("p (h w) -> p h w", w=52)
                conv(h2, CONV3, ws["w3"], bs["b3"],
                     lambda r0, R: h3v[:, r0 : r0 + R, :], AF.Relu)

                # ---- sparsify ----
                # 1) per-channel spatial max -> keep only entries == max
                nc.vector.tensor_reduce(rmax[:, :], h3[:, :], mybir.AxisListType.X, mybir.AluOpType.max)
                nc.vector.tensor_scalar(msk[:, :], h3[:, :], rmax[:, 0:1], None, mybir.AluOpType.is_equal)
                nc.vector.tensor_tensor(hm[:, :], h3[:, :], msk[:, :], mybir.AluOpType.mult)
                # 2) 4x4 block max over (channels x block positions)
                nc.vector.tensor_reduce(
                    bm1[:, :],
                    hm[:, :].rearrange("p (by dy x) -> p by x dy", by=13, dy=4),
                    mybir.AxisListType.X, mybir.AluOpType.max)
                nc.vector.tensor_reduce(
                    bm[:, :],
                    bm1[:, :].rearrange("p (by bx dx) -> p by bx dx", bx=13, dx=4),
                    mybir.AxisListType.X, mybir.AluOpType.max)
                # cross-channel max: PE transpose (channels -> free axis),
                # free-axis reduce, partition->free DMA, PE ones-broadcast
                pst1 = psp.tile([128, 512], F32, tag="ps", name="pst1")
                nc.tensor.transpose(pst1[:, 0:128], bm[:, 0:128], eye[:, :])
                pst2 = psp.tile([128, 512], F32, tag="ps", name="pst2")
                nc.tensor.transpose(pst2[0:41, 0:128], bm[:, 128:169], eye[:, :])
                nc.vector.tensor_reduce(rT[:, 0:1], pst1[:, 0:128], mybir.AxisListType.X, mybir.AluOpType.max)
                nc.vector.tensor_reduce(rT[0:41, 1:2], pst2[0:41, 0:128], mybir.AxisListType.X, mybir.AluOpType.max)
                nc.sync.dma_start(mb1[0:1, 0:128], rT[:, 0:1])
                nc.sync.dma_start(mb1[0:1, 128:169], rT[0:41, 1:2])
                psb = psp.tile([128, 512], F32, tag="ps", name="psb")
                nc.tensor.matmul(psb[:, :169], ones1[:, :], mb1[:, :], start=True, stop=True)
                nc.vector.tensor_copy(mb[:, :], psb[:, :169])
                # 3) keep entries equal to the block max (hm <= mb always)
                # keep entries equal to block max (hm <= mb always); ISA allows
                # only 3 free dims so split the block-broadcast compare over dy
                mbv = (mb[:, :].rearrange("p (by bx) -> p by bx", bx=13)
                       .unsqueeze(3).to_broadcast((128, 13, 13, 4)))
                hmv = hm[:, :].rearrange("p (by dy bx dx) -> p dy by bx dx", by=13, dy=4, bx=13, dx=4)
                mskv = msk[:, :].rearrange("p (by dy bx dx) -> p dy by bx dx", by=13, dy=4, bx=13, dx=4)
                for d in range(4):
                    nc.vector.tensor_tensor(
                        mskv[:, d], hmv[:, d], mbv, mybir.AluOpType.is_ge)
                # p1 aliases h1 (fully rewritten by conv1 each sample): re-zero
                # the 4-wide borders by DMA, then write the sparsified interior
                p1v = p1[:, :].rearrange("p (h w) -> p h w", w=60)
                nc.sync.dma_start(p1[:, 0:240], zc_d[:, 0:240])
                nc.sync.dma_start(p1[:, 3360:3600], zc_d[:, 0:240])
                nc.sync.dma_start(p1v[:, 4:56, 0:4],
                                  zc_d[:, 0:208].rearrange("p (h w) -> p h w", w=4))
                nc.sync.dma_start(p1v[:, 4:56, 56:60],
                                  zc_d[:, 0:208].rearrange("p (h w) -> p h w", w=4))
                nc.vector.tensor_tensor(
                    p1v[:, 4:56, 4:56],
                    hm[:, :].rearrange("p (h w) -> p h w", w=52),
                    msk[:, :].rearrange("p (h w) -> p h w", w=52),
                    mybir.AluOpType.mult)

                # ---- deconvs (convs over padded inputs, host-flipped weights) ----
                p2v = p2[:, :].rearrange("p (h w) -> p h w", w=64)
                conv(p1, DECONV1, ws["w4"], bs["b4"],
                     lambda r0, R: p2v[:, 4 + r0 : 4 + r0 + R, 4:60], AF.Relu)
                p3v = p3[:, :].rearrange("p (h w) -> p h w", w=68)
                conv(p2, DECONV2, ws["w5"], bs["b5"],
                     lambda r0, R: p3v[:, 4 + r0 : 4 + r0 + R, 4:64], AF.Relu)

                # ---- deconv3: 4 chunks run concurrently, one per PE column
                # group (out partitions 32g..32g+2); no cross-group combine
                p3iv = p3[:, :].rearrange("p (h w) -> p h w", w=68)
                for h in range(2):
                    ps = psp.tile([128, 512], F32, tag="ps", name="psd3")
                    for t, (dy, dx) in enumerate(TAPS):
                        for g in range(4):
                            r0 = (4 * h + g) * 8
                            nc.tensor.matmul(
                                ps[32 * g : 32 * g + 3, :],
                                w6[:, t, :],
                                p3iv[:, r0 + dy : r0 + dy + 8, dx : dx + 64],
                                start=(t == 0),
                                stop=(t == 24),
                                tile_position=(0, 32 * g),
                            )
                    for g in range(4):
                        nc.scalar.activation(
                            o_sb[32 * g : 32 * g + 3, 512 * h : 512 * h + 512],
                            ps[32 * g : 32 * g + 3, :],
                            AF.Sigmoid, bias=b6[32 * g : 32 * g + 3, 0:1])

                if b is None:
                    nc.vector.tensor_reduce(csum[:, 1:2], o_sb[:, :],
                                            mybir.AxisListType.X, mybir.AluOpType.max)
                    nc.vector.tensor_tensor(csum[:, 0:1], csum[:, 0:1], csum[:, 1:2],
                                            mybir.AluOpType.max)
                else:
                    # out[b, co, pix] <- o_sb[32g+co, 512h + n], pix = (4h+g)*512+n
                    ov = out_d[b].rearrange("co h w -> co (h w)")
                    for h in range(2):
                        for g in range(4):
                            c = 4 * h + g
                            nc.sync.dma_start(
                                ov[:, 512 * c : 512 * c + 512],
                                o_sb[32 * g : 32 * g + 3, 512 * h : 512 * h + 512],
                            )
            if repeat > 1:
                nc.sync.dma_start(csum_d[:, :], csum[0:3, 0:1])

    if split_waits:  # needed for HW walrus; CoreSim's race detector rejects it
        _split_waits(nc)
    nc.finalize()
    return nc


def _prep_weights(ew0, eb0, ew1, eb1, ew2, eb2, dw0, db0, dw1, db1, dw2, db2):
    f32 = lambda a: np.ascontiguousarray(a, dtype=np.float32)
    return {
        "w1": f32(ew0.transpose(2, 3, 1, 0).reshape(75, 128)),
        "w2": f32(ew1.transpose(1, 2, 3, 0).reshape(128, 25, 128)),
        "w3": f32(ew2.transpose(1, 2, 3, 0).reshape(128, 25, 128)),
        "w4": f32(dw0[:, :, ::-1, ::-1].transpose(0, 2, 3, 1).reshape(128, 25, 128)),
        "w5": f32(dw1[:, :, ::-1, ::-1].transpose(0, 2, 3, 1).reshape(128, 25, 128)),
        "w6": np.ascontiguousarray(
            dw2[:, :, ::-1, ::-1].transpose(0, 2, 3, 1).reshape(128, 25, 3)
        ).astype(_ml_bf16),
        "b1": f32(eb0.reshape(128, 1)),
        "b2": f32(eb1.reshape(128, 1)),
        "b3": f32(eb2.reshape(128, 1)),
        "b4": f32(db0.reshape(128, 1)),
        "b5": f32(db1.reshape(128, 1)),
        "b6": f32(np.concatenate([np.pad(db2.reshape(3), (0, 29)) for _ in range(4)])[:99].reshape(99, 1)),
        "eye": np.eye(128, dtype=np.float32),
        "zc": np.zeros((128, 272), np.float32),
    }


def _tf32_round(a):
    """Round-to-nearest-even to 10 mantissa bits (f32r/TF32-style)."""
    b = np.ascontiguousarray(a, np.float32).view(np.uint32)
    keep = np.uint32(0xFFFFE000)
    half = np.uint32(0x1000)
    lsb = (b >> np.uint32(13)) & np.uint32(1)
    out = (b + half - np.uint32(1) + lsb) & keep
    return out.view(np.float32)


_NC_CACHE = {}


def _get_nc(mm_dtype="f32"):
    if mm_dtype not in _NC_CACHE:
        _NC_CACHE[mm_dtype] = build(BPC, mm_dtype)
    return _NC_CACHE[mm_dtype]


def kernel(x, ew0, eb0, ew1, eb1, ew2, eb2, dw0, db0, dw1, db1, dw2, db2,
           mm_dtype="f32", trace=False):
    from concourse.bass_utils import run_bass_kernel_spmd

    nc = _get_nc(mm_dtype)
    wd = _prep_weights(ew0, eb0, ew1, eb1, ew2, eb2, dw0, db0, dw1, db1, dw2, db2)
    x = np.ascontiguousarray(x, dtype=np.float32)
    if mm_dtype == "f32r":
        for k in ("w4", "w5"):
            wd[k] = _tf32_round(wd[k])
    elif mm_dtype == "bf16d":
        for k in ("w4", "w5"):
            wd[k] = np.ascontiguousarray(wd[k]).astype(_ml_bf16)
        wd["zc"] = wd["zc"].astype(_ml_bf16)
    in_maps = [dict(wd, x=x[i * BPC : (i + 1) * BPC]) for i in range(N_CORES)]
    try:
        r = run_bass_kernel_spmd(nc, in_maps, core_ids=list(range(N_CORES)), trace=trace)
    except ModuleNotFoundError:
        r = run_bass_kernel_spmd(nc, in_maps, core_ids=list(range(N_CORES)), trace=False)
    out = np.concatenate([r.results[i]["out"] for i in range(N_CORES)], axis=0)
    if trace:
        kernel.last_result = r
    return out


# revision 88
# speedup vs baseline: 2.0826x; 2.0826x over previous
"""DeepConvAE Trainium2 kernel: 3x conv5x5+relu -> block-argmax sparsify ->
3x deconv5x5 (relu, relu, sigmoid). Data-parallel over batch: 64 samples
split 8-per-core across 8 NeuronCores; all weights replicated.

Convs are computed as 25-tap matmul accumulation in PSUM with input channels
on partitions (K), output channels as the stationary M dim, and output-pixel
row-chunks as the moving free dim N (<=512 per PSUM bank). Deconvs are convs
over zero-padded inputs with spatially-flipped / transposed weights (done on
host). conv1 (3 input channels) uses an im2col tile (K = 25 taps x 3 ch = 75)
built with strided DMAs straight from HBM.
"""

import sys

sys.path.insert(0, "/opt/trn_rl_repo")

import numpy as np

import bass_rust
import ml_dtypes
import concourse.bass as bass
import concourse.mybir as mybir

_ml_bf16 = np.dtype(ml_dtypes.bfloat16)
import concourse.tile as tile
from concourse import bass_isa
from concourse import library_config

F32 = mybir.dt.float32
F32R = mybir.dt.float32r
BF16 = mybir.dt.bfloat16

N_CORES = 8
BPC = 8  # batch per core
TAPS = [(dy, dx) for dy in range(5) for dx in range(5)]

# row-chunking per layer: (Hi, Wi, Ho, Wo, chunk row counts)
# all chunk N = R*Wo kept >= 256 so float32r streams at full rate
CONV1 = (64, 64, 60, 60, [8, 8, 8, 8, 7, 7, 7, 7])
CONV2 = (60, 60, 56, 56, [8, 8, 8, 8, 8, 8, 8])
CONV3 = (56, 56, 52, 52, [8, 8, 8, 7, 7, 7, 7])
DECONV1 = (60, 60, 56, 56, [8, 8, 8, 8, 8, 8, 8])  # input = padded 52+8
DECONV2 = (64, 64, 60, 60, [8, 8, 8, 8, 7, 7, 7, 7])  # input = padded 56+8
DECONV3 = (68, 68, 64, 64, [8, 8, 8, 8, 8, 8, 8, 8])  # input = padded 60+8


def _split_waits(nc):
    """This toolchain's walrus allows at most ONE sync-wait command per engine
    instruction ("Too many sync wait commands"). Tile emits one wait per
    upstream proc, so split: insert same-engine NOPs, each carrying one of the
    extra waits, immediately before the offending instruction."""
    for f in nc.m.functions:
        for blk in f.blocks:
            new = []
            for inst in blk.instructions:
                si = inst.sync_info
                if si is not None and si.on_wait is not None and len(si.on_wait) > 1:
                    waits = list(si.on_wait)
                    for k, w in enumerate(waits[:-1]):
                        nop = mybir.InstNoOp(name=f"{inst.name}_w{k}", ins=[], outs=[])
                        nop.engine = inst.engine
                        nop.sync_info = mybir.SyncInfo(on_wait=[w], on_update=[])
                        new.append(nop)
                    inst.sync_info = mybir.SyncInfo(
                        on_wait=[waits[-1]], on_update=list(si.on_update)
                    )
                new.append(inst)
            blk.instructions = new


def _r0s(chunks):
    r0, out = 0, []
    for r in chunks:
        out.append((r0, r))
        r0 += r
    return out


def build(n_samples=BPC, mm_dtype="f32r", split_waits=True, repeat=1):
    """Build the single-core Bass program (SPMD: same program on all cores)."""
    nc = bass.Bass()
    MMD = F32R if mm_dtype == "f32r" else F32
    DD = BF16 if mm_dtype == "bf16d" else MMD  # decoder (deconv1/2) dtype

    x_d = nc.dram_tensor("x", [n_samples, 3, 64, 64], F32, kind="ExternalInput")
    w1_d = nc.dram_tensor("w1", [75, 128], F32, kind="ExternalInput")
    w2_d = nc.dram_tensor("w2", [128, 25, 128], F32, kind="ExternalInput")
    w3_d = nc.dram_tensor("w3", [128, 25, 128], F32, kind="ExternalInput")
    w4_d = nc.dram_tensor("w4", [128, 25, 128], DD, kind="ExternalInput")
    w5_d = nc.dram_tensor("w5", [128, 25, 128], DD, kind="ExternalInput")
    w6_d = nc.dram_tensor("w6", [128, 25, 3], BF16, kind="ExternalInput")
    b1_d = nc.dram_tensor("b1", [128, 1], F32, kind="ExternalInput")
    b2_d = nc.dram_tensor("b2", [128, 1], F32, kind="ExternalInput")
    b3_d = nc.dram_tensor("b3", [128, 1], F32, kind="ExternalInput")
    b4_d = nc.dram_tensor("b4", [128, 1], F32, kind="ExternalInput")
    b5_d = nc.dram_tensor("b5", [128, 1], F32, kind="ExternalInput")
    b6_d = nc.dram_tensor("b6", [99, 1], F32, kind="ExternalInput")
    eye_d = nc.dram_tensor("eye", [128, 128], F32, kind="ExternalInput")
    zc_d = nc.dram_tensor("zc", [128, 272], DD, kind="ExternalInput")
    if repeat > 1:
        csum_d = nc.dram_tensor("csum", [3, 1], F32, kind="ExternalOutput")
    out_d = nc.dram_tensor("out", [n_samples, 3, 64, 64], F32, kind="ExternalOutput")

    AF = mybir.ActivationFunctionType

    with tile.TileContext(nc) as tc:
        with (
            tc.tile_pool(name="wp", bufs=1) as wp,
            tc.tile_pool(name="act", bufs=1) as ap_,
            tc.tile_pool(name="ps", bufs=8, space=bass.MemorySpace.PSUM) as psp,
        ):
            # ---- resident weights/biases ----
            w1 = wp.tile([75, 128], F32)
            nc.sync.dma_start(w1[:, :], w1_d[:, :])
            ws = {}
            for nm, d in (("w2", w2_d), ("w3", w3_d), ("w4", w4_d), ("w5", w5_d)):
                t = wp.tile([128, 25, 128], F32 if nm in ("w2", "w3") else DD, tag=nm)
                nc.sync.dma_start(t[:, :, :], d[:, :, :])
                ws[nm] = t
            w6 = wp.tile([128, 25, 3], BF16)
            nc.sync.dma_start(w6[:, :, :], w6_d[:, :, :])
            bs = {}
            for nm, d in (("b1", b1_d), ("b2", b2_d), ("b3", b3_d), ("b4", b4_d), ("b5", b5_d)):
                t = wp.tile([128, 1], F32, tag=nm)
                nc.sync.dma_start(t[:, :], d[:, :])
                bs[nm] = t
            b6 = wp.tile([99, 1], F32)
            nc.sync.dma_start(b6[:, :], b6_d[:, :])

            # ---- persistent activation tiles (reused across samples) ----
            c1in = ap_.tile([75, 3600], F32)   # conv1 im2col (tap,ci rows)
            h1 = ap_.tile([128, 3600], F32)    # 60x60; reused as padded p1
            h2 = ap_.tile([128, 3136], F32)    # 56x56
            h3 = ap_.tile([128, 2704], F32)    # 52x52
            hm = ap_.tile([128, 2704], F32)    # spatially-masked h3
            msk = ap_.tile([128, 2704], F32)   # scratch mask
            bm1 = ap_.tile([128, 676], F32)
            bm = ap_.tile([128, 169], F32)
            mb = ap_.tile([128, 169], F32)
            rT = ap_.tile([128, 2], F32)
            mb1 = ap_.tile([1, 169], F32)
            ones1 = ap_.tile([1, 128], F32)
            rmax = ap_.tile([128, 1], F32)
            nc.vector.memset(ones1[:, :], 1.0)
            eye = wp.tile([128, 128], F32)
            nc.sync.dma_start(eye[:, :], eye_d[:, :])
            p1 = ap_.tile([128, 3600], DD)     # padded 60x60 sparsify output
            p2 = ap_.tile([128, 4096], DD)     # padded 64x64
            p3 = ap_.tile([128, 4624], BF16)   # padded 68x68 (bf16 deconv3)
            o_sb = ap_.tile([99, 1024], F32)   # (group,co) x (half, 512)

            # zero pad borders (DMA from zero DRAM: memset can't emit f32r);
            # interiors are fully rewritten per sample
            def zero_borders(pt, W, zsrc):
                pv = pt[:, :].rearrange("p (h w) -> p h w", w=W)
                nc.sync.dma_start(pt[:, 0 : 4 * W], zsrc[:, 0 : 4 * W])
                nc.sync.dma_start(pt[:, (W - 4) * W : W * W], zsrc[:, 0 : 4 * W])
                nc.sync.dma_start(pv[:, 4 : W - 4, 0:4],
                                  zsrc[:, 0 : 4 * (W - 8)].rearrange("p (h w) -> p h w", w=4))
                nc.sync.dma_start(pv[:, 4 : W - 4, W - 4 : W],
                                  zsrc[:, 0 : 4 * (W - 8)].rearrange("p (h w) -> p h w", w=4))

            zero_borders(p2, 64, zc_d[:, :])
            zero_borders(p3, 68, zc_d[:, :] if DD == BF16 else zc_d[:, :].bitcast(BF16))
            if repeat > 1:
                csum = ap_.tile([99, 2], F32)
                nc.vector.memset(csum[:, :], 0.0)

            def conv(in_tile, geom, w, bias, dst_fn, func, M=128):
                Hi, Wi, Ho, Wo, chunks = geom
                iv = in_tile[:, :].rearrange("p (h w) -> p h w", w=Wi)
                for r0, R in _r0s(chunks):
                    N = R * Wo
                    ps = psp.tile([128, 512], F32, tag="ps", name="ps")
                    for t, (dy, dx) in enumerate(TAPS):
                        rhs = iv[:, r0 + dy : r0 + dy + R, dx : dx + Wo]
                        nc.tensor.matmul(
                            ps[:, :N],
                            w[:, t, :],
                            rhs,
                            start=(t == 0),
                            stop=(t == 24),
                        )
                    nc.scalar.activation(
                        dst_fn(r0, R),
                        ps[:, :N].rearrange("p (r w) -> p r w", w=Wo),
                        func,
                        bias=bias,
                    )

            # repeat>1 is a timing aid: first run the pipeline repeat-1 extra
            # times over existing inputs, folding results into a tiny checksum
            # output (keeps the work live without extra transfers); the real
            # n_samples iterations run last.
            warm = [(r % n_samples, None) for r in range(repeat - 1)]
            for bi, b in warm + [(b, b) for b in range(n_samples)]:
                # ---- conv1: im2col (75 = tap x ch rows), one matmul/chunk ----
                for t, (dy, dx) in enumerate(TAPS):
                    nc.sync.dma_start(
                        c1in[3 * t : 3 * t + 3, :],
                        x_d[bi, :, dy : dy + 60, dx : dx + 60],
                    )
                h1v = h1[:, :].rearrange("p (h w) -> p h w", w=60)
                for r0, R in _r0s(CONV1[4]):
                    N = R * 60
                    ps = psp.tile([128, 512], F32, tag="ps", name="ps")
                    nc.tensor.matmul(ps[:, :N], w1[:, :],
                                     c1in[:, r0 * 60 : (r0 + R) * 60],
                                     start=True, stop=True)
                    nc.scalar.activation(
                        h1v[:, r0 : r0 + R, :],
                        ps[:, :N].rearrange("p (r w) -> p r w", w=60),
                        AF.Relu, bias=bs["b1"])

                # ---- conv2, conv3 ----
                h2v = h2[:, :].rearrange("p (h w) -> p h w", w=56)
                conv(h1, CONV2, ws["w2"], bs["b2"],
                     lambda r0, R: h2v[:, r0 : r0 + R, :], AF.Relu)
                h3v = h3[:, :].rearrange("p (h w) -> p h w", w=52)
                conv(h2, CONV3, ws["w3"], bs["b3"],
                     lambda r0, R: h3v[:, r0 : r0 + R, :], AF.Relu)

                # ---- sparsify ----
                # 1) per-channel spatial max -> keep only entries == max
                nc.vector.tensor_reduce(rmax[:, :], h3[:, :], mybir.AxisListType.X, mybir.AluOpType.max)
                nc.vector.tensor_scalar(msk[:, :], h3[:, :], rmax[:, 0:1], None, mybir.AluOpType.is_equal)
                nc.vector.tensor_tensor(hm[:, :], h3[:, :], msk[:, :], mybir.AluOpType.mult)
                # 2) 4x4 block max over (channels x block positions)
                nc.vector.tensor_reduce(
                    bm1[:, :],
                    hm[:, :].rearrange("p (by dy x) -> p by x dy", by=13, dy=4),
                    mybir.AxisListType.X, mybir.AluOpType.max)
                nc.vector.tensor_reduce(
                    bm[:, :],
                    bm1[:, :].rearrange("p (by bx dx) -> p by bx dx", bx=13, dx=4),
                    mybir.AxisListType.X, mybir.AluOpType.max)
                # cross-channel max: PE transpose (channels -> free axis),
                # free-axis reduce, partition->free DMA, PE ones-broadcast
                pst1 = psp.tile([128, 512], F32, tag="ps", name="pst1")
                nc.tensor.transpose(pst1[:, 0:128], bm[:, 0:128], eye[:, :])
                pst2 = psp.tile([128, 512], F32, tag="ps", name="pst2")
                nc.tensor.transpose(pst2[0:41, 0:128], bm[:, 128:169], eye[:, :])
                nc.vector.tensor_reduce(rT[:, 0:1], pst1[:, 0:128], mybir.AxisListType.X, mybir.AluOpType.max)
                nc.vector.tensor_reduce(rT[0:41, 1:2], pst2[0:41, 0:128], mybir.AxisListType.X, mybir.AluOpType.max)
                nc.sync.dma_start(mb1[0:1, 0:128], rT[:, 0:1])
                nc.sync.dma_start(mb1[0:1, 128:169], rT[0:41, 1:2])
                psb = psp.tile([128, 512], F32, tag="ps", name="psb")
                nc.tensor.matmul(psb[:, :169], ones1[:, :], mb1[:, :], start=True, stop=True)
                nc.vector.tensor_copy(mb[:, :], psb[:, :169])
                # 3) keep entries equal to the block max (hm <= mb always)
                # keep entries equal to block max (hm <= mb always); ISA allows
                # only 3 free dims so split the block-broadcast compare over dy
                mbv = (mb[:, :].rearrange("p (by bx) -> p by bx", bx=13)
                       .unsqueeze(3).to_broadcast((128, 13, 13, 4)))
                hmv = hm[:, :].rearrange("p (by dy bx dx) -> p dy by bx dx", by=13, dy=4, bx=13, dx=4)
                mskv = msk[:, :].rearrange("p (by dy bx dx) -> p dy by bx dx", by=13, dy=4, bx=13, dx=4)
                for d in range(4):
                    nc.vector.tensor_tensor(
                        mskv[:, d], hmv[:, d], mbv, mybir.AluOpType.is_ge)
                # p1 aliases h1 (fully rewritten by conv1 each sample): re-zero
                # the 4-wide borders by DMA, then write the sparsified interior
                p1v = p1[:, :].rearrange("p (h w) -> p h w", w=60)
                nc.sync.dma_start(p1[:, 0:240], zc_d[:, 0:240])
                nc.sync.dma_start(p1[:, 3360:3600], zc_d[:, 0:240])
                nc.sync.dma_start(p1v[:, 4:56, 0:4],
                                  zc_d[:, 0:208].rearrange("p (h w) -> p h w", w=4))
                nc.sync.dma_start(p1v[:, 4:56, 56:60],
                                  zc_d[:, 0:208].rearrange("p (h w) -> p h w", w=4))
                nc.vector.tensor_tensor(
                    p1v[:, 4:56, 4:56],
                    hm[:, :].rearrange("p (h w) -> p h w", w=52),
                    msk[:, :].rearrange("p (h w) -> p h w", w=52),
                    mybir.AluOpType.mult)

                # ---- deconvs (convs over padded inputs, host-flipped weights) ----
                p2v = p2[:, :].rearrange("p (h w) -> p h w", w=64)
                conv(p1, DECONV1, ws["w4"], bs["b4"],
                     lambda r0, R: p2v[:, 4 + r0 : 4 + r0 + R, 4:60], AF.Relu)
                p3v = p3[:, :].rearrange("p (h w) -> p h w", w=68)
                conv(p2, DECONV2, ws["w5"], bs["b5"],
                     lambda r0, R: p3v[:, 4 + r0 : 4 + r0 + R, 4:64], AF.Relu)

                # ---- deconv3: 4 chunks run concurrently, one per PE column
                # group (out partitions 32g..32g+2); no cross-group combine
                p3iv = p3[:, :].rearrange("p (h w) -> p h w", w=68)
                for h in range(2):
                    ps = psp.tile([128, 512], F32, tag="ps", name="psd3")
                    for t, (dy, dx) in enumerate(TAPS):
                        for g in range(4):
                            r0 = (4 * h + g) * 8
                            nc.tensor.matmul(
                                ps[32 * g : 32 * g + 3, :],
                                w6[:, t, :],
                                p3iv[:, r0 + dy : r0 + dy + 8, dx : dx + 64],
                                start=(t == 0),
                                stop=(t == 24),
                                tile_position=(0, 32 * g),
                            )
                    for g in range(4):
                        nc.scalar.activation(
                            o_sb[32 * g : 32 * g + 3, 512 * h : 512 * h + 512],
                            ps[32 * g : 32 * g + 3, :],
                            AF.Sigmoid, bias=b6[32 * g : 32 * g + 3, 0:1])

                if b is None:
                    nc.vector.tensor_reduce(csum[:, 1:2], o_sb[:, :],
                                            mybir.AxisListType.X, mybir.AluOpType.max)
                    nc.vector.tensor_tensor(csum[:, 0:1], csum[:, 0:1], csum[:, 1:2],
                                            mybir.AluOpType.max)
                else:
                    # out[b, co, pix] <- o_sb[32g+co, 512h + n], pix = (4h+g)*512+n
                    ov = out_d[b].rearrange("co h w -> co (h w)")
                    for h in range(2):
                        for g in range(4):
                            c = 4 * h + g
                            nc.sync.dma_start(
                                ov[:, 512 * c : 512 * c + 512],
                                o_sb[32 * g : 32 * g + 3, 512 * h : 512 * h + 512],
                            )
            if repeat > 1:
                nc.sync.dma_start(csum_d[:, :], csum[0:3, 0:1])

    if split_waits:  # needed for HW walrus; CoreSim's race detector rejects it
        _split_waits(nc)
    nc.finalize()
    return nc


def _prep_weights(ew0, eb0, ew1, eb1, ew2, eb2, dw0, db0, dw1, db1, dw2, db2):
    f32 = lambda a: np.ascontiguousarray(a, dtype=np.float32)
    return {
        "w1": f32(ew0.transpose(2, 3, 1, 0).reshape(75, 128)),
        "w2": f32(ew1.transpose(1, 2, 3, 0).reshape(128, 25, 128)),
        "w3": f32(ew2.transpose(1, 2, 3, 0).reshape(128, 25, 128)),
        "w4": f32(dw0[:, :, ::-1, ::-1].transpose(0, 2, 3, 1).reshape(128, 25, 128)),
        "w5": f32(dw1[:, :, ::-1, ::-1].transpose(0, 2, 3, 1).reshape(128, 25, 128)),
        "w6": np.ascontiguousarray(
            dw2[:, :, ::-1, ::-1].transpose(0, 2, 3, 1).reshape(128, 25, 3)
        ).astype(_ml_bf16),
        "b1": f32(eb0.reshape(128, 1)),
        "b2": f32(eb1.reshape(128, 1)),
        "b3": f32(eb2.reshape(128, 1)),
        "b4": f32(db0.reshape(128, 1)),
        "b5": f32(db1.reshape(128, 1)),
        "b6": f32(np.concatenate([np.pad(db2.reshape(3), (0, 29)) for _ in range(4)])[:99].reshape(99, 1)),
        "eye": np.eye(128, dtype=np.float32),
        "zc": np.zeros((128, 272), np.float32),
    }


def _tf32_round(a):
    """Round-to-nearest-even to 10 mantissa bits (f32r/TF32-style)."""
    b = np.ascontiguousarray(a, np.float32).view(np.uint32)
    keep = np.uint32(0xFFFFE000)
    half = np.uint32(0x1000)
    lsb = (b >> np.uint32(13)) & np.uint32(1)
    out = (b + half - np.uint32(1) + lsb) & keep
    return out.view(np.float32)


_NC_CACHE = {}


def _get_nc(mm_dtype="bf16d"):
    if mm_dtype not in _NC_CACHE:
        _NC_CACHE[mm_dtype] = build(BPC, mm_dtype)
    return _NC_CACHE[mm_dtype]


def kernel(x, ew0, eb0, ew1, eb1, ew2, eb2, dw0, db0, dw1, db1, dw2, db2,
           mm_dtype="bf16d", trace=False):
    from concourse.bass_utils import run_bass_kernel_spmd

    nc = _get_nc(mm_dtype)
    wd = _prep_weights(ew0, eb0, ew1, eb1, ew2, eb2, dw0, db0, dw1, db1, dw2, db2)
    x = np.ascontiguousarray(x, dtype=np.float32)
    if mm_dtype == "f32r":
        for k in ("w4", "w5"):
            wd[k] = _tf32_round(wd[k])
    elif mm_dtype == "bf16d":
        for k in ("w4", "w5"):
            wd[k] = np.ascontiguousarray(wd[k]).astype(_ml_bf16)
        wd["zc"] = wd["zc"].astype(_ml_bf16)
    in_maps = [dict(wd, x=x[i * BPC : (i + 1) * BPC]) for i in range(N_CORES)]
    try:
        r = run_bass_kernel_spmd(nc, in_maps, core_ids=list(range(N_CORES)), trace=trace)
    except ModuleNotFoundError:
        r = run_bass_kernel_spmd(nc, in_maps, core_ids=list(range(N_CORES)), trace=False)
    out = np.concatenate([r.results[i]["out"] for i in range(N_CORES)], axis=0)
    if trace:
        kernel.last_result = r
    return out
